# revision 14
# baseline (speedup 1.0000x reference)
"""Trainium2 Bass kernel for nn_Composer (gnn_message_passing).

Math (exact reformulation of the reference):
  out[b,s1,:] = (heads[b,s1]==0) * ( base + sum_{s2: heads[b,s2]==s1} w[s2]*(t_on[b,s2]-t_off) )
  t_on[b,s2]  = tanh(u[b,s2] + bc),  u[b,s2,o] = tok[b,s2] @ Wc[o] @ tanh(tok[b,s2])
  t_off       = tanh(bc),  base = t_off*sum(w) + br

Only rows s2 whose head lands on a row with head==0 contribute to the output,
so u is needed for a handful of rows (R ~ 4-16 of 4096). The unavoidable cost
is streaming the 226 MB bilinear weight Wc once. Sharding: Wc is split over
the output dim O=384 across 8 cores (48 each, 28.3 MB/core); every core
computes its o-slice of u for all selected rows via 3 accumulated matmuls per
output channel (contraction d on partitions, Wc streamed as the moving
operand), then a fused multiply+reduce against dep on the vector engine.
The host does index selection, sharding, and the final scatter of the ~R
result vectors into the zero output.
"""
import numpy as np

import concourse.bass as bass
import concourse.bacc as bacc
import concourse.mybir as mybir
from concourse.tile import TileContext
from concourse.tile_rust import add_dep_helper
from concourse.bass_utils import run_bass_kernel_spmd

F32 = mybir.dt.float32
F32R = mybir.dt.float32r

B, S, D = 8, 512, 384
NCORES = 8
OC = D // NCORES          # output channels per core = 48
DC = D // 128             # contraction chunks = 3
R_MAX = 64                # padded selected-row capacity per device run
O_PER_DMA = 2             # output channels per Wc DMA (tile = 1.18 MB)
N_GRP = OC // O_PER_DMA
WC_BUFS = 4

_nc_cache = {}


def _build_nc():
    if "nc" in _nc_cache:
        return _nc_cache["nc"]
    nc = bacc.Bacc("TRN2", target_bir_lowering=False, debug=False)
    wc_d = nc.dram_tensor("wc", [N_GRP, 128, O_PER_DMA * DC * 384], F32R,
                          kind="ExternalInput")
    tokT_d = nc.dram_tensor("tokT", [128, DC * R_MAX], F32R, kind="ExternalInput")
    tok_d = nc.dram_tensor("tok", [R_MAX, D], F32, kind="ExternalInput")
    w_d = nc.dram_tensor("w", [R_MAX, 1], F32, kind="ExternalInput")
    bcr_d = nc.dram_tensor("bcrep", [128, OC], F32, kind="ExternalInput")
    contrib_d = nc.dram_tensor("contrib", [R_MAX, OC], F32, kind="ExternalOutput")
    toff_d = nc.dram_tensor("toff", [1, OC], F32, kind="ExternalOutput")

    AF = mybir.ActivationFunctionType
    OP = mybir.AluOpType

    with TileContext(nc) as tc:
        with (
            tc.tile_pool(name="const", bufs=1) as cp,
            tc.tile_pool(name="wcp", bufs=WC_BUFS) as wcp,
            tc.tile_pool(name="zp", bufs=48) as zp,
            tc.tile_pool(name="pp", bufs=4, space="PSUM") as pp,
        ):
            tokT_sb = cp.tile([128, DC * R_MAX], F32R)
            nc.sync.dma_start(out=tokT_sb[:], in_=tokT_d[:])
            tok_sb = cp.tile([R_MAX, D], F32)
            nc.sync.dma_start(out=tok_sb[:], in_=tok_d[:])
            w_sb = cp.tile([R_MAX, 1], F32)
            nc.sync.dma_start(out=w_sb[:], in_=w_d[:])
            bcr_sb = cp.tile([128, OC], F32)
            nc.sync.dma_start(out=bcr_sb[:], in_=bcr_d[:])

            dep_sb = cp.tile([R_MAX, D], F32)
            nc.scalar.activation(dep_sb[:], tok_sb[:], AF.Tanh)
            toff_sb = cp.tile([128, OC], F32)
            nc.scalar.activation(toff_sb[:], bcr_sb[:], AF.Tanh)
            # DVE observes dep's ACT tick here so per-o reduce ops carry only
            # the PE wait (walrus allows a single sync wait per instruction);
            # same for the w/bcr DMA lanes used by the epilogue.
            dep_touch = cp.tile([R_MAX, 1], F32)
            nc.vector.tensor_copy(out=dep_touch[:], in_=dep_sb[:, 0:1])
            w_touch = cp.tile([R_MAX, 1], F32)
            nc.vector.tensor_copy(out=w_touch[:], in_=w_sb[:])
            bcr_touch = cp.tile([128, 1], F32)
            nc.vector.tensor_copy(out=bcr_touch[:], in_=bcr_sb[:, 0:1])

            u_sb = cp.tile([R_MAX, OC], F32)
            for g in range(N_GRP):
                wt = wcp.tile([128, O_PER_DMA * DC * 384], F32R, tag="wc")
                nc.sync.dma_start(out=wt[:], in_=wc_d[g])
                for oi in range(O_PER_DMA):
                    o = g * O_PER_DMA + oi
                    ps = pp.tile([R_MAX, 384], F32, tag="ps")
                    for c in range(DC):
                        nc.tensor.matmul(
                            ps[:],
                            lhsT=tokT_sb[:, c * R_MAX:(c + 1) * R_MAX],
                            rhs=wt[:, (oi * DC + c) * 384:(oi * DC + c + 1) * 384],
                            start=(c == 0), stop=(c == DC - 1),
                        )
                    z = zp.tile([R_MAX, 384], F32, tag="z")
                    nc.vector.scalar_tensor_tensor(
                        out=z[:], in0=ps[:], scalar=1.0, in1=dep_sb[:],
                        op0=OP.mult, op1=OP.mult,
                        accum_out=u_sb[:, o:o + 1],
                    )

            ton_sb = cp.tile([R_MAX, OC], F32)
            nc.vector.tensor_tensor(ton_sb[:], u_sb[:], bcr_sb[0:R_MAX, :], OP.add)
            nc.scalar.activation(ton_sb[:], ton_sb[:], AF.Tanh)
            c_sb = cp.tile([R_MAX, OC], F32)
            nc.vector.tensor_tensor(c_sb[:], ton_sb[:], toff_sb[0:R_MAX, :],
                                    OP.subtract)
            nc.vector.tensor_scalar_mul(c_sb[:], c_sb[:], w_sb[:])
            nc.sync.dma_start(out=contrib_d[:], in_=c_sb[:])
            nc.sync.dma_start(out=toff_d[:], in_=toff_sb[0:1, :])

    nc.compile()
    _nc_cache["nc"] = nc
    return nc


def _shard_wc(Wc):
    """Per-core Wc layout: [N_GRP, 128(p), O_PER_DMA*DC*384] with
    o = g*O_PER_DMA + oi, d = c*128 + p, free index = (oi*DC + c)*384 + e."""
    shards = []
    for k in range(NCORES):
        wck = Wc[k * OC:(k + 1) * OC]                       # [48, 384, 384]
        wck = wck.reshape(N_GRP, O_PER_DMA, DC, 128, 384)
        wck = np.ascontiguousarray(wck.transpose(0, 3, 1, 2, 4))
        shards.append(wck.reshape(N_GRP, 128, O_PER_DMA * DC * 384))
    return shards


def run_device(in_maps, trace=False, tmpdir=None):
    nc = _build_nc()
    return run_bass_kernel_spmd(nc, in_maps, list(range(NCORES)),
                                trace=trace, tmpdir=tmpdir)


def _make_in_maps(tok_sel, w_sel, wc_shards, bc):
    """tok_sel [R_MAX, D] f32, w_sel [R_MAX] f32."""
    # tokT[p, c*R_MAX + r] = tok_sel[r, c*128 + p]
    tokT = np.ascontiguousarray(
        tok_sel.T.reshape(DC, 128, R_MAX).transpose(1, 0, 2)
    ).reshape(128, DC * R_MAX)
    maps = []
    for k in range(NCORES):
        maps.append({
            "wc": wc_shards[k],
            "tokT": tokT,
            "tok": tok_sel,
            "w": w_sel.reshape(R_MAX, 1),
            "bcrep": np.ascontiguousarray(
                np.broadcast_to(bc[k * OC:(k + 1) * OC], (128, OC))),
        })
    return maps


def kernel(**inputs):
    tokens = np.asarray(inputs["tokens"])
    heads = np.asarray(inputs["dep_heads"])
    tok_table = np.asarray(inputs["tok_table"], dtype=np.float32)
    Wc = np.asarray(inputs["Wc"], dtype=np.float32)
    bc = np.asarray(inputs["bc"], dtype=np.float32)
    Wr = np.asarray(inputs["Wr"], dtype=np.float32)
    br = np.asarray(inputs["br"], dtype=np.float32)
    assert tokens.shape == (B, S) and Wc.shape == (D, D, D)

    # host index selection: rows that can reach an unmasked (head==0) output row
    zs = [np.nonzero(heads[b] == 0)[0] for b in range(B)]
    sel = [(b, int(s2), int(heads[b, s2]))
           for b in range(B)
           for s2 in np.nonzero(np.isin(heads[b], zs[b]))[0]]
    R = len(sel)

    wc_shards = _shard_wc(Wc)
    w_full = Wr[0]

    contribs = []
    toff = None
    for lo in range(0, max(R, 1), R_MAX):
        chunk = sel[lo:lo + R_MAX]
        tok_sel = np.zeros((R_MAX, D), dtype=np.float32)
        w_sel = np.zeros(R_MAX, dtype=np.float32)
        for i, (b, s2, _dest) in enumerate(chunk):
            tok_sel[i] = tok_table[tokens[b, s2]]
            w_sel[i] = w_full[s2]
        res = run_device(_make_in_maps(tok_sel, w_sel, wc_shards, bc)).results
        contribs.append(np.concatenate(
            [res[k]["contrib"] for k in range(NCORES)], axis=1))
        toff = np.concatenate([res[k]["toff"][0] for k in range(NCORES)])

    base = (toff * w_full.sum() + br[0]).astype(np.float32)
    out = np.zeros((B, S, D), dtype=np.float32)
    for b in range(B):
        out[b, zs[b]] = base
    for i, (b, _s2, dest) in enumerate(sel):
        out[b, dest] += contribs[i // R_MAX][i % R_MAX]
    return out


# revision 20
# speedup vs baseline: 1.0227x; 1.0227x over previous
"""Trainium2 Bass kernel for nn_Composer (gnn_message_passing).

Math (exact reformulation of the reference):
  out[b,s1,:] = (heads[b,s1]==0) * ( base + sum_{s2: heads[b,s2]==s1} w[s2]*(t_on[b,s2]-t_off) )
  t_on[b,s2]  = tanh(u[b,s2] + bc),  u[b,s2,o] = tok[b,s2] @ Wc[o] @ tanh(tok[b,s2])
  t_off       = tanh(bc),  base = t_off*sum(w) + br

Only rows s2 whose head lands on a row with head==0 contribute to the output,
so u is needed for a handful of rows (R ~ 4-16 of 4096). The unavoidable cost
is streaming the 226 MB bilinear weight Wc once. Sharding: Wc is split over
the output dim O=384 across 8 cores (48 each, 28.3 MB/core); every core
computes its o-slice of u for all selected rows via 3 accumulated matmuls per
output channel (contraction d on partitions, Wc streamed as the moving
operand), then a fused multiply+reduce against dep on the vector engine.
The host does index selection, sharding, and the final scatter of the ~R
result vectors into the zero output.
"""
import numpy as np

import concourse.bass as bass
import concourse.bacc as bacc
import concourse.mybir as mybir
from concourse.tile import TileContext
from concourse.tile_rust import add_dep_helper
from concourse.bass_utils import run_bass_kernel_spmd

F32 = mybir.dt.float32
F32R = mybir.dt.float32r

B, S, D = 8, 512, 384
NCORES = 8
OC = D // NCORES          # output channels per core = 48
DC = D // 128             # contraction chunks = 3
R_MAX = 64                # padded selected-row capacity per device run
O_PER_DMA = 3             # output channels per Wc DMA (tile = 1.77 MB)
N_GRP = OC // O_PER_DMA
WC_BUFS = 5

_nc_cache = {}


def _build_nc():
    if "nc" in _nc_cache:
        return _nc_cache["nc"]
    nc = bacc.Bacc("TRN2", target_bir_lowering=False, debug=False)
    wc_d = nc.dram_tensor("wc", [N_GRP, 128, O_PER_DMA * DC * 384], F32R,
                          kind="ExternalInput")
    tokT_d = nc.dram_tensor("tokT", [128, DC * R_MAX], F32R, kind="ExternalInput")
    tok_d = nc.dram_tensor("tok", [R_MAX, D], F32, kind="ExternalInput")
    w_d = nc.dram_tensor("w", [R_MAX, 1], F32, kind="ExternalInput")
    bcr_d = nc.dram_tensor("bcrep", [128, OC], F32, kind="ExternalInput")
    contrib_d = nc.dram_tensor("contrib", [R_MAX, OC], F32, kind="ExternalOutput")
    toff_d = nc.dram_tensor("toff", [1, OC], F32, kind="ExternalOutput")

    AF = mybir.ActivationFunctionType
    OP = mybir.AluOpType

    HOC = OC // 2             # 24 output channels per epilogue half
    HGRP = N_GRP // 2

    with TileContext(nc) as tc:
        with (
            tc.tile_pool(name="const", bufs=1) as cp,
            tc.tile_pool(name="wcp", bufs=WC_BUFS) as wcp,
            tc.tile_pool(name="zp", bufs=16) as zp,
            tc.tile_pool(name="pp", bufs=4, space="PSUM") as pp,
        ):
            # kick off the Wc stream first — it is the critical path
            wts = []
            for g in range(N_GRP):
                wts.append(wcp.tile([128, O_PER_DMA * DC * 384], F32R, tag="wc",
                                    name=f"wt{g}"))
            for g in range(WC_BUFS):
                nc.sync.dma_start(out=wts[g][:], in_=wc_d[g])

            tokT_sb = cp.tile([128, DC * R_MAX], F32R)
            nc.sync.dma_start(out=tokT_sb[:], in_=tokT_d[:])
            tok_sb = cp.tile([R_MAX, D], F32)
            nc.sync.dma_start(out=tok_sb[:], in_=tok_d[:])
            w_sb = cp.tile([R_MAX, 1], F32)
            nc.sync.dma_start(out=w_sb[:], in_=w_d[:])
            bcr_sb = cp.tile([128, OC], F32)
            nc.sync.dma_start(out=bcr_sb[:], in_=bcr_d[:])

            dep_sb = cp.tile([R_MAX, D], F32)
            nc.scalar.activation(dep_sb[:], tok_sb[:], AF.Tanh)
            toff_sb = cp.tile([128, OC], F32)
            nc.scalar.activation(toff_sb[:], bcr_sb[:], AF.Tanh)
            nc.sync.dma_start(out=toff_d[:], in_=toff_sb[0:1, :])
            # DVE observes dep/w/bcr ticks here so the hot-loop reduce ops
            # carry few sync waits (each extra wait costs an event semaphore)
            dep_touch = cp.tile([R_MAX, 1], F32)
            nc.vector.tensor_copy(out=dep_touch[:], in_=dep_sb[:, 0:1])
            # toffw[r,o] = tanh(bc)[o] * w[r], independent of u — compute early
            toffw_sb = cp.tile([R_MAX, OC], F32)
            nc.vector.tensor_scalar_mul(toffw_sb[:], toff_sb[0:R_MAX, :], w_sb[:])

            u_half = [cp.tile([R_MAX, HOC], F32, tag="u0", name="u0"),
                      cp.tile([R_MAX, HOC], F32, tag="u1", name="u1")]

            def epilogue_half(h):
                ton = cp.tile([R_MAX, HOC], F32, tag=f"ton{h}", name=f"ton{h}")
                nc.vector.tensor_tensor(
                    ton[:], u_half[h][:],
                    bcr_sb[0:R_MAX, h * HOC:(h + 1) * HOC], OP.add)
                nc.scalar.activation(ton[:], ton[:], AF.Tanh)
                csb = cp.tile([R_MAX, HOC], F32, tag=f"c{h}", name=f"c{h}")
                # contrib = t_on*w - t_off*w
                nc.vector.scalar_tensor_tensor(
                    out=csb[:], in0=ton[:], scalar=w_sb[:],
                    in1=toffw_sb[:, h * HOC:(h + 1) * HOC],
                    op0=OP.mult, op1=OP.subtract)
                nc.sync.dma_start(out=contrib_d[:, h * HOC:(h + 1) * HOC],
                                  in_=csb[:])

            for g in range(N_GRP):
                if g >= WC_BUFS:
                    nc.sync.dma_start(out=wts[g][:], in_=wc_d[g])
                wt = wts[g]
                for oi in range(O_PER_DMA):
                    o = g * O_PER_DMA + oi
                    ps = pp.tile([R_MAX, 384], F32, tag="ps")
                    for c in range(DC):
                        nc.tensor.matmul(
                            ps[:],
                            lhsT=tokT_sb[:, c * R_MAX:(c + 1) * R_MAX],
                            rhs=wt[:, (oi * DC + c) * 384:(oi * DC + c + 1) * 384],
                            start=(c == 0), stop=(c == DC - 1),
                        )
                    z = zp.tile([R_MAX, 384], F32, tag="z")
                    nc.vector.scalar_tensor_tensor(
                        out=z[:], in0=ps[:], scalar=1.0, in1=dep_sb[:],
                        op0=OP.mult, op1=OP.mult,
                        accum_out=u_half[o // HOC][:, o % HOC:o % HOC + 1],
                    )
                if g == HGRP - 1:
                    epilogue_half(0)
            epilogue_half(1)

    nc.compile()
    _nc_cache["nc"] = nc
    return nc


def _shard_wc(Wc):
    """Per-core Wc layout: [N_GRP, 128(p), O_PER_DMA*DC*384] with
    o = g*O_PER_DMA + oi, d = c*128 + p, free index = (oi*DC + c)*384 + e."""
    shards = []
    for k in range(NCORES):
        wck = Wc[k * OC:(k + 1) * OC]                       # [48, 384, 384]
        wck = wck.reshape(N_GRP, O_PER_DMA, DC, 128, 384)
        wck = np.ascontiguousarray(wck.transpose(0, 3, 1, 2, 4))
        shards.append(wck.reshape(N_GRP, 128, O_PER_DMA * DC * 384))
    return shards


def run_device(in_maps, trace=False, tmpdir=None):
    nc = _build_nc()
    return run_bass_kernel_spmd(nc, in_maps, list(range(NCORES)),
                                trace=trace, tmpdir=tmpdir)


def _make_in_maps(tok_sel, w_sel, wc_shards, bc):
    """tok_sel [R_MAX, D] f32, w_sel [R_MAX] f32."""
    # tokT[p, c*R_MAX + r] = tok_sel[r, c*128 + p]
    tokT = np.ascontiguousarray(
        tok_sel.T.reshape(DC, 128, R_MAX).transpose(1, 0, 2)
    ).reshape(128, DC * R_MAX)
    maps = []
    for k in range(NCORES):
        maps.append({
            "wc": wc_shards[k],
            "tokT": tokT,
            "tok": tok_sel,
            "w": w_sel.reshape(R_MAX, 1),
            "bcrep": np.ascontiguousarray(
                np.broadcast_to(bc[k * OC:(k + 1) * OC], (128, OC))),
        })
    return maps


def kernel(**inputs):
    tokens = np.asarray(inputs["tokens"])
    heads = np.asarray(inputs["dep_heads"])
    tok_table = np.asarray(inputs["tok_table"], dtype=np.float32)
    Wc = np.asarray(inputs["Wc"], dtype=np.float32)
    bc = np.asarray(inputs["bc"], dtype=np.float32)
    Wr = np.asarray(inputs["Wr"], dtype=np.float32)
    br = np.asarray(inputs["br"], dtype=np.float32)
    assert tokens.shape == (B, S) and Wc.shape == (D, D, D)

    # host index selection: rows that can reach an unmasked (head==0) output row
    zs = [np.nonzero(heads[b] == 0)[0] for b in range(B)]
    sel = [(b, int(s2), int(heads[b, s2]))
           for b in range(B)
           for s2 in np.nonzero(np.isin(heads[b], zs[b]))[0]]
    R = len(sel)

    wc_shards = _shard_wc(Wc)
    w_full = Wr[0]

    contribs = []
    toff = None
    for lo in range(0, max(R, 1), R_MAX):
        chunk = sel[lo:lo + R_MAX]
        tok_sel = np.zeros((R_MAX, D), dtype=np.float32)
        w_sel = np.zeros(R_MAX, dtype=np.float32)
        for i, (b, s2, _dest) in enumerate(chunk):
            tok_sel[i] = tok_table[tokens[b, s2]]
            w_sel[i] = w_full[s2]
        res = run_device(_make_in_maps(tok_sel, w_sel, wc_shards, bc)).results
        contribs.append(np.concatenate(
            [res[k]["contrib"] for k in range(NCORES)], axis=1))
        toff = np.concatenate([res[k]["toff"][0] for k in range(NCORES)])

    base = (toff * w_full.sum() + br[0]).astype(np.float32)
    out = np.zeros((B, S, D), dtype=np.float32)
    for b in range(B):
        out[b, zs[b]] = base
    for i, (b, _s2, dest) in enumerate(sel):
        out[b, dest] += contribs[i // R_MAX][i % R_MAX]
    return out


# revision 26
# speedup vs baseline: 1.0850x; 1.0609x over previous
"""Trainium2 Bass kernel for nn_Composer (gnn_message_passing).

Math (exact reformulation of the reference):
  out[b,s1,:] = (heads[b,s1]==0) * ( base + sum_{s2: heads[b,s2]==s1} w[s2]*(t_on[b,s2]-t_off) )
  t_on[b,s2]  = tanh(u[b,s2] + bc),  u[b,s2,o] = tok[b,s2] @ Wc[o] @ tanh(tok[b,s2])
  t_off       = tanh(bc),  base = t_off*sum(w) + br

Only rows s2 whose head lands on a row with head==0 contribute to the output,
so u is needed for a handful of rows (R ~ 4-16 of 4096). The unavoidable cost
is streaming the 226 MB bilinear weight Wc once. Sharding: Wc is split over
the output dim O=384 across 8 cores (48 each, 28.3 MB/core); every core
computes its o-slice of u for all selected rows via 3 accumulated matmuls per
output channel (contraction d on partitions, Wc streamed as the moving
operand), then a fused multiply+reduce against dep on the vector engine.
The host does index selection, sharding, and the final scatter of the ~R
result vectors into the zero output.
"""
import numpy as np

import concourse.bass as bass
import concourse.bacc as bacc
import concourse.mybir as mybir
from concourse.tile import TileContext
from concourse.tile_rust import add_dep_helper
from concourse.bass_utils import run_bass_kernel_spmd

F32 = mybir.dt.float32
F32R = mybir.dt.float32r

B, S, D = 8, 512, 384
NCORES = 8
OC = D // NCORES          # output channels per core = 48
DC = D // 128             # contraction chunks = 3
R_MAX = 64                # padded selected-row capacity per device run
# Wc transfer group sizes (in output channels): small head groups so compute
# starts early, big middle groups for DMA efficiency, small tail groups so the
# final DMA->compute->epilogue chain is short.
GROUP_SIZES = [1, 1, 1] + [3] * 14 + [1, 1, 1]
assert sum(GROUP_SIZES) == OC
N_GRP = len(GROUP_SIZES)
WC_BUFS = 6

_nc_cache = {}


def _build_nc():
    if "nc" in _nc_cache:
        return _nc_cache["nc"]
    nc = bacc.Bacc("TRN2", target_bir_lowering=False, debug=False)
    wc_d = nc.dram_tensor("wc", [OC, 128, DC * 384], F32R,
                          kind="ExternalInput")
    tokT_d = nc.dram_tensor("tokT", [128, DC * R_MAX], F32R, kind="ExternalInput")
    tok_d = nc.dram_tensor("tok", [R_MAX, D], F32, kind="ExternalInput")
    w_d = nc.dram_tensor("w", [R_MAX, 1], F32, kind="ExternalInput")
    bcr_d = nc.dram_tensor("bcrep", [128, OC], F32, kind="ExternalInput")
    contrib_d = nc.dram_tensor("contrib", [R_MAX, OC], F32, kind="ExternalOutput")
    toff_d = nc.dram_tensor("toff", [1, OC], F32, kind="ExternalOutput")

    AF = mybir.ActivationFunctionType
    OP = mybir.AluOpType

    HOC = OC // 2             # 24 output channels per epilogue half

    with TileContext(nc) as tc:
        with (
            tc.tile_pool(name="const", bufs=1) as cp,
            tc.tile_pool(name="wcp", bufs=WC_BUFS) as wcp,
            tc.tile_pool(name="zp", bufs=16) as zp,
            tc.tile_pool(name="pp", bufs=4, space="PSUM") as pp,
        ):
            offs = [sum(GROUP_SIZES[:g]) for g in range(N_GRP)]

            def wc_dma(g, wt):
                no = GROUP_SIZES[g]
                nc.sync.dma_start(
                    out=wt[:].rearrange("p (o f) -> p o f", o=no),
                    in_=wc_d[offs[g]:offs[g] + no].rearrange("o p f -> p o f"))

            # Wc stream owns the SP HWDGE ring; everything small goes through
            # the scalar engine's ring so it never queues behind megabytes.
            wts = []
            for g in range(N_GRP):
                wts.append(wcp.tile([128, GROUP_SIZES[g] * DC * 384], F32R,
                                    tag="wc", name=f"wt{g}"))
            for g in range(WC_BUFS):
                wc_dma(g, wts[g])

            tokT_sb = cp.tile([128, DC * R_MAX], F32R)
            nc.scalar.dma_start(out=tokT_sb[:], in_=tokT_d[:])
            tok_sb = cp.tile([R_MAX, D], F32)
            nc.scalar.dma_start(out=tok_sb[:], in_=tok_d[:])
            w_sb = cp.tile([R_MAX, 1], F32)
            nc.scalar.dma_start(out=w_sb[:], in_=w_d[:])
            bcr_sb = cp.tile([128, OC], F32)
            nc.scalar.dma_start(out=bcr_sb[:], in_=bcr_d[:])

            dep_sb = cp.tile([R_MAX, D], F32)
            nc.scalar.activation(dep_sb[:], tok_sb[:], AF.Tanh)
            toff_sb = cp.tile([128, OC], F32)
            nc.scalar.activation(toff_sb[:], bcr_sb[:], AF.Tanh)
            nc.scalar.dma_start(out=toff_d[:], in_=toff_sb[0:1, :])
            # DVE observes dep/w/bcr ticks here so the hot-loop reduce ops
            # carry few sync waits (each extra wait costs an event semaphore)
            dep_touch = cp.tile([R_MAX, 1], F32)
            nc.vector.tensor_copy(out=dep_touch[:], in_=dep_sb[:, 0:1])
            # toffw[r,o] = tanh(bc)[o] * w[r], independent of u — compute early
            toffw_sb = cp.tile([R_MAX, OC], F32)
            nc.vector.tensor_scalar_mul(toffw_sb[:], toff_sb[0:R_MAX, :], w_sb[:])

            u_half = [cp.tile([R_MAX, HOC], F32, tag="u0", name="u0"),
                      cp.tile([R_MAX, HOC], F32, tag="u1", name="u1")]

            def epilogue_half(h):
                ton = cp.tile([R_MAX, HOC], F32, tag=f"ton{h}", name=f"ton{h}")
                nc.vector.tensor_tensor(
                    ton[:], u_half[h][:],
                    bcr_sb[0:R_MAX, h * HOC:(h + 1) * HOC], OP.add)
                nc.scalar.activation(ton[:], ton[:], AF.Tanh)
                csb = cp.tile([R_MAX, HOC], F32, tag=f"c{h}", name=f"c{h}")
                # contrib = t_on*w - t_off*w
                nc.vector.scalar_tensor_tensor(
                    out=csb[:], in0=ton[:], scalar=w_sb[:],
                    in1=toffw_sb[:, h * HOC:(h + 1) * HOC],
                    op0=OP.mult, op1=OP.subtract)
                nc.scalar.dma_start(out=contrib_d[:, h * HOC:(h + 1) * HOC],
                                    in_=csb[:])

            for g in range(N_GRP):
                if g >= WC_BUFS:
                    wc_dma(g, wts[g])
                wt = wts[g]
                for oi in range(GROUP_SIZES[g]):
                    o = offs[g] + oi
                    ps = pp.tile([R_MAX, 384], F32, tag="ps")
                    for c in range(DC):
                        nc.tensor.matmul(
                            ps[:],
                            lhsT=tokT_sb[:, c * R_MAX:(c + 1) * R_MAX],
                            rhs=wt[:, (oi * DC + c) * 384:(oi * DC + c + 1) * 384],
                            start=(c == 0), stop=(c == DC - 1),
                        )
                    z = zp.tile([R_MAX, 384], F32, tag="z")
                    nc.vector.scalar_tensor_tensor(
                        out=z[:], in0=ps[:], scalar=1.0, in1=dep_sb[:],
                        op0=OP.mult, op1=OP.mult,
                        accum_out=u_half[o // HOC][:, o % HOC:o % HOC + 1],
                    )
                    if o == HOC - 1:
                        epilogue_half(0)
            epilogue_half(1)

    nc.compile()
    _nc_cache["nc"] = nc
    return nc


def _shard_wc(Wc):
    """Per-core Wc layout: [OC, 128(p), DC*384] with d = c*128 + p,
    free index = c*384 + e."""
    shards = []
    for k in range(NCORES):
        wck = Wc[k * OC:(k + 1) * OC]                       # [48, 384, 384]
        wck = wck.reshape(OC, DC, 128, 384)
        wck = np.ascontiguousarray(wck.transpose(0, 2, 1, 3))
        shards.append(wck.reshape(OC, 128, DC * 384))
    return shards


def run_device(in_maps, trace=False, tmpdir=None):
    nc = _build_nc()
    return run_bass_kernel_spmd(nc, in_maps, list(range(NCORES)),
                                trace=trace, tmpdir=tmpdir)


def _make_in_maps(tok_sel, w_sel, wc_shards, bc):
    """tok_sel [R_MAX, D] f32, w_sel [R_MAX] f32."""
    # tokT[p, c*R_MAX + r] = tok_sel[r, c*128 + p]
    tokT = np.ascontiguousarray(
        tok_sel.T.reshape(DC, 128, R_MAX).transpose(1, 0, 2)
    ).reshape(128, DC * R_MAX)
    maps = []
    for k in range(NCORES):
        maps.append({
            "wc": wc_shards[k],
            "tokT": tokT,
            "tok": tok_sel,
            "w": w_sel.reshape(R_MAX, 1),
            "bcrep": np.ascontiguousarray(
                np.broadcast_to(bc[k * OC:(k + 1) * OC], (128, OC))),
        })
    return maps


def kernel(**inputs):
    tokens = np.asarray(inputs["tokens"])
    heads = np.asarray(inputs["dep_heads"])
    tok_table = np.asarray(inputs["tok_table"], dtype=np.float32)
    Wc = np.asarray(inputs["Wc"], dtype=np.float32)
    bc = np.asarray(inputs["bc"], dtype=np.float32)
    Wr = np.asarray(inputs["Wr"], dtype=np.float32)
    br = np.asarray(inputs["br"], dtype=np.float32)
    assert tokens.shape == (B, S) and Wc.shape == (D, D, D)

    # host index selection: rows that can reach an unmasked (head==0) output row
    zs = [np.nonzero(heads[b] == 0)[0] for b in range(B)]
    sel = [(b, int(s2), int(heads[b, s2]))
           for b in range(B)
           for s2 in np.nonzero(np.isin(heads[b], zs[b]))[0]]
    R = len(sel)

    wc_shards = _shard_wc(Wc)
    w_full = Wr[0]

    contribs = []
    toff = None
    for lo in range(0, max(R, 1), R_MAX):
        chunk = sel[lo:lo + R_MAX]
        tok_sel = np.zeros((R_MAX, D), dtype=np.float32)
        w_sel = np.zeros(R_MAX, dtype=np.float32)
        for i, (b, s2, _dest) in enumerate(chunk):
            tok_sel[i] = tok_table[tokens[b, s2]]
            w_sel[i] = w_full[s2]
        res = run_device(_make_in_maps(tok_sel, w_sel, wc_shards, bc)).results
        contribs.append(np.concatenate(
            [res[k]["contrib"] for k in range(NCORES)], axis=1))
        toff = np.concatenate([res[k]["toff"][0] for k in range(NCORES)])

    base = (toff * w_full.sum() + br[0]).astype(np.float32)
    out = np.zeros((B, S, D), dtype=np.float32)
    for b in range(B):
        out[b, zs[b]] = base
    for i, (b, _s2, dest) in enumerate(sel):
        out[b, dest] += contribs[i // R_MAX][i % R_MAX]
    return out


# revision 29
# speedup vs baseline: 1.2350x; 1.1382x over previous
"""Trainium2 Bass kernel for nn_Composer (gnn_message_passing).

Math (exact reformulation of the reference):
  out[b,s1,:] = (heads[b,s1]==0) * ( base + sum_{s2: heads[b,s2]==s1} w[s2]*(t_on[b,s2]-t_off) )
  t_on[b,s2]  = tanh(u[b,s2] + bc),  u[b,s2,o] = tok[b,s2] @ Wc[o] @ tanh(tok[b,s2])
  t_off       = tanh(bc),  base = t_off*sum(w) + br

Only rows s2 whose head lands on a row with head==0 contribute to the output,
so u is needed for a handful of rows (R ~ 4-16 of 4096). The unavoidable cost
is streaming the 226 MB bilinear weight Wc once. Sharding: Wc is split over
the output dim O=384 across 8 cores (48 each, 28.3 MB/core); every core
computes its o-slice of u for all selected rows via 3 accumulated matmuls per
output channel (contraction d on partitions, Wc streamed as the moving
operand), then a fused multiply+reduce against dep on the vector engine.
The host does index selection, sharding, and the final scatter of the ~R
result vectors into the zero output.
"""
import numpy as np

import concourse.bass as bass
import concourse.bacc as bacc
import concourse.mybir as mybir
from concourse.tile import TileContext
from concourse.tile_rust import add_dep_helper
from concourse.bass_utils import run_bass_kernel_spmd

F32 = mybir.dt.float32
F32R = mybir.dt.float32r

B, S, D = 8, 512, 384
NCORES = 8
OC = D // NCORES          # output channels per core = 48
DC = D // 128             # contraction chunks = 3
R_MAX = 64                # padded selected-row capacity per device run
# Wc transfer group sizes (in output channels): small head groups so compute
# starts early, big middle groups for DMA efficiency, small tail groups so the
# final DMA->compute->epilogue chain is short.
GROUP_SIZES = [1, 2] + [3] * 14 + [2, 1]
assert sum(GROUP_SIZES) == OC
N_GRP = len(GROUP_SIZES)
WC_BUFS = 6

_nc_cache = {}


def _build_nc():
    if "nc" in _nc_cache:
        return _nc_cache["nc"]
    nc = bacc.Bacc("TRN2", target_bir_lowering=False, debug=False)
    wc_d = nc.dram_tensor("wc", [OC, 128, DC * 384], F32R,
                          kind="ExternalInput")
    tokT_d = nc.dram_tensor("tokT", [128, DC * R_MAX], F32R, kind="ExternalInput")
    tok_d = nc.dram_tensor("tok", [R_MAX, D], F32, kind="ExternalInput")
    w_d = nc.dram_tensor("w", [R_MAX, 1], F32, kind="ExternalInput")
    bcr_d = nc.dram_tensor("bcrep", [128, OC], F32, kind="ExternalInput")
    contrib_d = nc.dram_tensor("contrib", [R_MAX, OC], F32, kind="ExternalOutput")
    toff_d = nc.dram_tensor("toff", [1, OC], F32, kind="ExternalOutput")

    AF = mybir.ActivationFunctionType
    OP = mybir.AluOpType

    HOC = OC // 2             # 24 output channels per epilogue half

    with TileContext(nc) as tc:
        with (
            tc.tile_pool(name="const", bufs=1) as cp,
            tc.tile_pool(name="wcp", bufs=WC_BUFS) as wcp,
            tc.tile_pool(name="zp", bufs=16) as zp,
            tc.tile_pool(name="pp", bufs=4, space="PSUM") as pp,
        ):
            offs = [sum(GROUP_SIZES[:g]) for g in range(N_GRP)]

            def wc_dma(g, wt):
                no = GROUP_SIZES[g]
                nc.sync.dma_start(
                    out=wt[:].rearrange("p (o f) -> p o f", o=no),
                    in_=wc_d[offs[g]:offs[g] + no].rearrange("o p f -> p o f"))

            # Wc stream owns the SP HWDGE ring; everything small goes through
            # the scalar engine's ring so it never queues behind megabytes.
            wts = []
            for g in range(N_GRP):
                wts.append(wcp.tile([128, GROUP_SIZES[g] * DC * 384], F32R,
                                    tag="wc", name=f"wt{g}"))
            for g in range(WC_BUFS):
                wc_dma(g, wts[g])

            tokT_sb = cp.tile([128, DC * R_MAX], F32R)
            nc.scalar.dma_start(out=tokT_sb[:], in_=tokT_d[:])
            tok_sb = cp.tile([R_MAX, D], F32)
            nc.scalar.dma_start(out=tok_sb[:], in_=tok_d[:])
            w_sb = cp.tile([R_MAX, 1], F32)
            nc.scalar.dma_start(out=w_sb[:], in_=w_d[:])
            bcr_sb = cp.tile([128, OC], F32)
            nc.scalar.dma_start(out=bcr_sb[:], in_=bcr_d[:])

            dep_sb = cp.tile([R_MAX, D], F32)
            nc.scalar.activation(dep_sb[:], tok_sb[:], AF.Tanh)
            toff_sb = cp.tile([128, OC], F32)
            nc.scalar.activation(toff_sb[:], bcr_sb[:], AF.Tanh)
            nc.scalar.dma_start(out=toff_d[:], in_=toff_sb[0:1, :])
            # DVE observes dep/w/bcr ticks here so the hot-loop reduce ops
            # carry few sync waits (each extra wait costs an event semaphore)
            dep_touch = cp.tile([R_MAX, 1], F32)
            nc.vector.tensor_copy(out=dep_touch[:], in_=dep_sb[:, 0:1])
            # toffw[r,o] = tanh(bc)[o] * w[r], independent of u — compute early
            toffw_sb = cp.tile([R_MAX, OC], F32)
            nc.vector.tensor_scalar_mul(toffw_sb[:], toff_sb[0:R_MAX, :], w_sb[:])

            u_half = [cp.tile([R_MAX, HOC], F32, tag="u0", name="u0"),
                      cp.tile([R_MAX, HOC], F32, tag="u1", name="u1")]

            def epilogue(lo, hi):
                """contrib[:, lo:hi] = w*(tanh(u+bc) - t_off). For a single
                channel the +bc folds into the ACT bias port (bc is constant
                across partitions), skipping the DVE add."""
                n = hi - lo
                ton = cp.tile([R_MAX, n], F32, tag=f"ton{lo}", name=f"ton{lo}")
                uv = (u_half[0][:, lo:hi] if hi <= HOC
                      else u_half[1][:, lo - HOC:hi - HOC])
                if n == 1:
                    nc.scalar.activation(ton[:], uv, AF.Tanh,
                                         bias=bcr_sb[0:R_MAX, lo:lo + 1])
                else:
                    nc.vector.tensor_tensor(ton[:], uv,
                                            bcr_sb[0:R_MAX, lo:hi], OP.add)
                    nc.scalar.activation(ton[:], ton[:], AF.Tanh)
                csb = cp.tile([R_MAX, n], F32, tag=f"c{lo}", name=f"c{lo}")
                # contrib = t_on*w - t_off*w
                nc.vector.scalar_tensor_tensor(
                    out=csb[:], in0=ton[:], scalar=w_sb[:],
                    in1=toffw_sb[:, lo:hi],
                    op0=OP.mult, op1=OP.subtract)
                nc.scalar.dma_start(out=contrib_d[:, lo:hi], in_=csb[:])

            for g in range(N_GRP):
                if g >= WC_BUFS:
                    wc_dma(g, wts[g])
                wt = wts[g]
                for oi in range(GROUP_SIZES[g]):
                    o = offs[g] + oi
                    ps = pp.tile([R_MAX, 384], F32, tag="ps")
                    for c in range(DC):
                        nc.tensor.matmul(
                            ps[:],
                            lhsT=tokT_sb[:, c * R_MAX:(c + 1) * R_MAX],
                            rhs=wt[:, (oi * DC + c) * 384:(oi * DC + c + 1) * 384],
                            start=(c == 0), stop=(c == DC - 1),
                        )
                    z = zp.tile([R_MAX, 384], F32, tag="z")
                    nc.vector.scalar_tensor_tensor(
                        out=z[:], in0=ps[:], scalar=1.0, in1=dep_sb[:],
                        op0=OP.mult, op1=OP.mult,
                        accum_out=u_half[o // HOC][:, o % HOC:o % HOC + 1],
                    )
                    if o == HOC - 1:
                        epilogue(0, HOC)
                    elif o == OC - 2:
                        epilogue(HOC, OC - 1)
            epilogue(OC - 1, OC)

    nc.compile()
    _nc_cache["nc"] = nc
    return nc


def _shard_wc(Wc):
    """Per-core Wc layout: [OC, 128(p), DC*384] with d = c*128 + p,
    free index = c*384 + e."""
    shards = []
    for k in range(NCORES):
        wck = Wc[k * OC:(k + 1) * OC]                       # [48, 384, 384]
        wck = wck.reshape(OC, DC, 128, 384)
        wck = np.ascontiguousarray(wck.transpose(0, 2, 1, 3))
        shards.append(wck.reshape(OC, 128, DC * 384))
    return shards


def run_device(in_maps, trace=False, tmpdir=None):
    nc = _build_nc()
    return run_bass_kernel_spmd(nc, in_maps, list(range(NCORES)),
                                trace=trace, tmpdir=tmpdir)


def _make_in_maps(tok_sel, w_sel, wc_shards, bc):
    """tok_sel [R_MAX, D] f32, w_sel [R_MAX] f32."""
    # tokT[p, c*R_MAX + r] = tok_sel[r, c*128 + p]
    tokT = np.ascontiguousarray(
        tok_sel.T.reshape(DC, 128, R_MAX).transpose(1, 0, 2)
    ).reshape(128, DC * R_MAX)
    maps = []
    for k in range(NCORES):
        maps.append({
            "wc": wc_shards[k],
            "tokT": tokT,
            "tok": tok_sel,
            "w": w_sel.reshape(R_MAX, 1),
            "bcrep": np.ascontiguousarray(
                np.broadcast_to(bc[k * OC:(k + 1) * OC], (128, OC))),
        })
    return maps


def kernel(**inputs):
    tokens = np.asarray(inputs["tokens"])
    heads = np.asarray(inputs["dep_heads"])
    tok_table = np.asarray(inputs["tok_table"], dtype=np.float32)
    Wc = np.asarray(inputs["Wc"], dtype=np.float32)
    bc = np.asarray(inputs["bc"], dtype=np.float32)
    Wr = np.asarray(inputs["Wr"], dtype=np.float32)
    br = np.asarray(inputs["br"], dtype=np.float32)
    assert tokens.shape == (B, S) and Wc.shape == (D, D, D)

    # host index selection: rows that can reach an unmasked (head==0) output row
    zs = [np.nonzero(heads[b] == 0)[0] for b in range(B)]
    sel = [(b, int(s2), int(heads[b, s2]))
           for b in range(B)
           for s2 in np.nonzero(np.isin(heads[b], zs[b]))[0]]
    R = len(sel)

    wc_shards = _shard_wc(Wc)
    w_full = Wr[0]

    contribs = []
    toff = None
    for lo in range(0, max(R, 1), R_MAX):
        chunk = sel[lo:lo + R_MAX]
        tok_sel = np.zeros((R_MAX, D), dtype=np.float32)
        w_sel = np.zeros(R_MAX, dtype=np.float32)
        for i, (b, s2, _dest) in enumerate(chunk):
            tok_sel[i] = tok_table[tokens[b, s2]]
            w_sel[i] = w_full[s2]
        res = run_device(_make_in_maps(tok_sel, w_sel, wc_shards, bc)).results
        contribs.append(np.concatenate(
            [res[k]["contrib"] for k in range(NCORES)], axis=1))
        toff = np.concatenate([res[k]["toff"][0] for k in range(NCORES)])

    base = (toff * w_full.sum() + br[0]).astype(np.float32)
    out = np.zeros((B, S, D), dtype=np.float32)
    for b in range(B):
        out[b, zs[b]] = base
    for i, (b, _s2, dest) in enumerate(sel):
        out[b, dest] += contribs[i // R_MAX][i % R_MAX]
    return out
